# revision 1
# baseline (speedup 1.0000x reference)
"""Causal self-attention on 8 Trainium2 NeuronCores (Bass/Tile).

Problem: x[4, 2048, 1024], w_in[3072, 1024], w_out[1024, 1024], 16 heads.
    qkv = x @ w_in.T ; per-(b,h) causal softmax attention ; out = y @ w_out.T

Sharding (SPMD — one program, per-core input data):
    core c  ->  batch b = c // 2, head-group g = c % 2 (heads 8g .. 8g+7).
    Each core projects q/k/v for its 8 heads of its batch and runs causal
    attention for them.  The pair (2b, 2b+1) AllGathers the two head-group
    halves of yT (chunked per head-pair so it overlaps attention), then each
    core computes the output projection for half of the output features
    (core even: e_out 0..511, odd: 512..1023) over all 2048 tokens of its
    batch.  The host concatenates.

Everything on-chip is kept feature-major ("T" = contraction dim on SBUF
partitions) so no on-device transposes are needed:
    xT [D, S] (host-transposed), qT/kT per head-pair [128, S],
    scoresT [k, q], yT [e, t], outT [e_out, t] (host-transposed back).
Head-pair q/k projections are interleaved with that pair's attention so the
PE stays dense while the ACT engine works through the exps.  Softmax
denominators come from a ones-column appended to V (the AV matmul has
M = 65); normalization is a fast DVE reciprocal on the sum row (moved to
partition 0 by a small DMA — custom DVE ops only work at base partition 0)
+ a K=1 matmul broadcast + one multiply, streamed straight to DRAM.
Matmuls run as float32r (reduced-precision fp32, full PE rate at N >= 256).
"""

import sys

for _p in ("/opt/trn_rl_repo",):
    if _p not in sys.path:
        sys.path.insert(0, _p)

import numpy as np

B, S, D = 4, 2048, 1024
H, HD = 16, 64
N_CORES = 8
HPC = 8            # heads per core
NPAIRS = HPC // 2  # head pairs per core
QC = S // 512      # q-chunks per head
TT = S // 128      # token tiles
DT = D // 128      # feature (d) tiles
EHALF = D // 2     # output features per core

_PROG = None       # cached compiled program


def _build_program():
    import concourse.bass as bass
    from concourse import bacc
    import concourse.tile as tile
    import concourse.mybir as mybir
    from contextlib import ExitStack

    f32 = mybir.dt.float32
    f32r = mybir.dt.float32r
    AF = mybir.ActivationFunctionType
    OP = mybir.AluOpType

    nc = bacc.Bacc("TRN2", target_bir_lowering=False, debug=False,
                   num_devices=N_CORES)

    xT = nc.dram_tensor("xT", [D, S], f32r, kind="ExternalInput").ap()
    wqkT = nc.dram_tensor("wqkT", [D, 2 * HPC * HD], f32r,
                          kind="ExternalInput").ap()
    wvT = nc.dram_tensor("wvT", [D, HPC * HD], f32r, kind="ExternalInput").ap()
    woT = nc.dram_tensor("woT", [D, EHALF], f32r, kind="ExternalInput").ap()
    tri = nc.dram_tensor("tri", [128, 128], f32, kind="ExternalInput").ap()
    outT = nc.dram_tensor("outT", [EHALF, S], f32, kind="ExternalOutput").ap()

    y_loc = nc.dram_tensor("y_loc", [HPC * HD, S], f32r)
    y_gat = [nc.dram_tensor(f"y_gat{i}", [2, 128, S], f32r)
             for i in range(NPAIRS)]

    with tile.TileContext(nc) as tc:
        def mm(out, lhsT, rhs, start, stop):
            nc.tensor.matmul(out, lhsT, rhs, start=start, stop=stop)

        with ExitStack() as perm:
            const_pool = perm.enter_context(tc.tile_pool(name="const", bufs=1))
            v_pool = perm.enter_context(tc.tile_pool(name="vsb", bufs=TT))
            mm_ps = perm.enter_context(
                tc.tile_pool(name="mmps", bufs=2, space="PSUM"))

            tri_sb = const_pool.tile([128, 128], f32, tag="tri")
            nc.sync.dma_start(tri_sb[:], tri[:])
            ones_sb = const_pool.tile([128, 64], f32, tag="ones")
            nc.gpsimd.memset(ones_sb[:], 1.0)
            onesr_sb = const_pool.tile([1, 64], f32r, tag="onesr")
            nc.vector.tensor_copy(onesr_sb[:], ones_sb[0:1, :])

            # v_sb[t]: [128, 8*65] — per head 64 v-columns + a ones column
            v_sb = [v_pool.tile([128, HPC * (HD + 1)], f32r, tag="v",
                                name=f"v{t}") for t in range(TT)]

            with ExitStack() as att_scope:
                qk_pool = att_scope.enter_context(
                    tc.tile_pool(name="qksb", bufs=4))
                xt_pool = att_scope.enter_context(
                    tc.tile_pool(name="xtsb", bufs=DT))
                wqk_pool = att_scope.enter_context(
                    tc.tile_pool(name="wqksb", bufs=2 * DT))
                p_pool = att_scope.enter_context(
                    tc.tile_pool(name="psb", bufs=3))
                n_pool = att_scope.enter_context(
                    tc.tile_pool(name="nsb", bufs=2))
                sc_ps = att_scope.enter_context(
                    tc.tile_pool(name="scps", bufs=2, space="PSUM"))
                y_ps = att_scope.enter_context(
                    tc.tile_pool(name="yps", bufs=2, space="PSUM"))

                xt_sb = [xt_pool.tile([128, S], f32r, tag="xt", name=f"xt{d}")
                         for d in range(DT)]
                for d in range(DT):
                    nc.sync.dma_start(xt_sb[d][:], xT[d * 128:(d + 1) * 128, :])

                # ---- v projection: v[t, e] accumulated over d ----
                with tc.tile_pool(name="wvsb", bufs=DT) as wv_pool:
                    wv_sb = [wv_pool.tile([128, HPC * HD], f32r, tag="wv",
                                          name=f"wv{d}") for d in range(DT)]
                    for d in range(DT):
                        nc.sync.dma_start(wv_sb[d][:],
                                          wvT[d * 128:(d + 1) * 128, :])
                    for t in range(TT):
                        ps = mm_ps.tile([128, 512], f32, tag="mm")
                        for d in range(DT):
                            mm(ps[:], xt_sb[d][:, t * 128:(t + 1) * 128],
                               wv_sb[d][:], start=(d == 0), stop=(d == DT - 1))
                        vdst = v_sb[t][:].rearrange(
                            "p (h e) -> p h e", h=HPC)[:, :, 0:HD]
                        vsrc = ps[:].rearrange("p (h e) -> p h e", h=HPC)
                        nc.vector.tensor_copy(vdst, vsrc)
                        nc.vector.tensor_copy(
                            v_sb[t][:].rearrange(
                                "p (h e) -> p h e", h=HPC)[:, :, HD:HD + 1],
                            ones_sb[:, 0:HPC].unsqueeze(-1))

                # ---- per pair: q/k projection then attention ----
                for i in range(NPAIRS):
                    wqk_sb = [wqk_pool.tile([128, 256], f32r, tag="wqk",
                                            name=f"wqk{i}_{d}")
                              for d in range(DT)]
                    for d in range(DT):
                        nc.sync.dma_start(
                            wqk_sb[d][:, 0:128],
                            wqkT[d * 128:(d + 1) * 128, i * 128:(i + 1) * 128])
                        nc.sync.dma_start(
                            wqk_sb[d][:, 128:256],
                            wqkT[d * 128:(d + 1) * 128,
                                 (NPAIRS + i) * 128:(NPAIRS + i + 1) * 128])
                    q_sb = qk_pool.tile([128, S], f32r, tag="qk", name=f"q{i}")
                    k_sb = qk_pool.tile([128, S], f32r, tag="qk", name=f"k{i}")
                    for which, dest in ((0, q_sb), (1, k_sb)):
                        for qc in range(QC):
                            ps = mm_ps.tile([128, 512], f32, tag="mm")
                            for d in range(DT):
                                mm(ps[:],
                                   wqk_sb[d][:, which * 128:(which + 1) * 128],
                                   xt_sb[d][:, qc * 512:(qc + 1) * 512],
                                   start=(d == 0), stop=(d == DT - 1))
                            nc.vector.tensor_copy(
                                dest[:, qc * 512:(qc + 1) * 512], ps[:])

                    # ---- attention for this pair ----
                    for qc in range(QC):
                        nkt = 4 * qc + 4   # causal: k-tiles 0 .. 4qc+3
                        yps = [y_ps.tile([65, 512], f32, tag="yt",
                                         name=f"yps{i}_{qc}_{h}")
                               for h in range(2)]
                        for kt in range(nkt):
                            j = kt - 4 * qc
                            lo = max(0, j) * 128
                            sc = sc_ps.tile([128, 1024], f32, tag="sc")
                            pt = p_pool.tile([128, 1024], f32r, tag="p")
                            for h in range(2):
                                mm(sc[:, h * 512 + lo:(h + 1) * 512],
                                   k_sb[h * 64:(h + 1) * 64,
                                        kt * 128:(kt + 1) * 128],
                                   q_sb[h * 64:(h + 1) * 64,
                                        qc * 512 + lo:(qc + 1) * 512],
                                   start=True, stop=True)
                            # exp(score / 8) for both heads in one ACT call
                            src = sc[:].rearrange("p (s c) -> p s c", s=2)[
                                :, :, lo:512]
                            dst = pt[:].rearrange("p (s c) -> p s c", s=2)[
                                :, :, lo:512]
                            nc.scalar.activation(dst, src, AF.Exp, scale=0.125)
                            if j >= 0:   # mask the diagonal band
                                for h in range(2):
                                    band = pt[:, h * 512 + lo:
                                              h * 512 + lo + 128]
                                    nc.vector.tensor_mul(band, band, tri_sb[:])
                            for h in range(2):
                                hl = 2 * i + h
                                mm(yps[h][:, lo:512],
                                   v_sb[kt][:, hl * 65:hl * 65 + 65],
                                   pt[:, h * 512 + lo:(h + 1) * 512],
                                   start=(kt == 0), stop=(kt == nkt - 1))
                        # normalize: y[0:64] * (1 / y[64]) and stream to DRAM
                        for h in range(2):
                            ysc = n_pool.tile([65, 512], f32, tag="ysc")
                            nc.vector.tensor_copy(ysc[:], yps[h][:])
                            srow = n_pool.tile([1, 512], f32, tag="srow")
                            nc.sync.dma_start(srow[:], ysc[64:65, :])
                            rcp = n_pool.tile([1, 512], f32, tag="rcp")
                            nc.vector.reciprocal_approx_fast(
                                out=rcp[:], in_=srow[:])
                            rcpr = n_pool.tile([1, 512], f32r, tag="rcpr")
                            nc.vector.tensor_copy(rcpr[:], rcp[:])
                            rb = mm_ps.tile([64, 512], f32, tag="mm")
                            mm(rb[:], onesr_sb[:], rcpr[:],
                               start=True, stop=True)
                            nout = n_pool.tile([64, 512], f32r, tag="nout")
                            nc.vector.tensor_mul(nout[:], ysc[0:64, :], rb[:])
                            nc.sync.dma_start(
                                y_loc[(2 * i + h) * 64:(2 * i + h + 1) * 64,
                                      qc * 512:(qc + 1) * 512], nout[:])

                    # ---- chunked pair AllGather for this head-pair ----
                    nc.gpsimd.collective_compute(
                        "AllGather", OP.bypass,
                        replica_groups=[[0, 1], [2, 3], [4, 5], [6, 7]],
                        ins=[y_loc[i * 128:(i + 1) * 128, :]],
                        outs=[y_gat[i][:]])

            # ---------------- output projection ----------------
            with ExitStack() as oproj:
                wo_pool = oproj.enter_context(
                    tc.tile_pool(name="wosb", bufs=DT))
                yg_pool = oproj.enter_context(
                    tc.tile_pool(name="ygsb", bufs=DT))
                o_pool = oproj.enter_context(tc.tile_pool(name="osb", bufs=2))

                wo_sb = [wo_pool.tile([128, EHALF], f32r, tag="wo",
                                      name=f"wo{d}") for d in range(DT)]
                for d in range(DT):
                    nc.sync.dma_start(wo_sb[d][:],
                                      woT[d * 128:(d + 1) * 128, :])
                yg_sb = [yg_pool.tile([128, S], f32r, tag="yg", name=f"yg{k}")
                         for k in range(DT)]
                for k in range(DT):
                    nc.sync.dma_start(yg_sb[k][:], y_gat[k % NPAIRS][k // NPAIRS])

                for m in range(EHALF // 128):
                    for tch in range(QC):
                        ps = mm_ps.tile([128, 512], f32, tag="mm")
                        for k in range(DT):
                            mm(ps[:], wo_sb[k][:, m * 128:(m + 1) * 128],
                               yg_sb[k][:, tch * 512:(tch + 1) * 512],
                               start=(k == 0), stop=(k == DT - 1))
                        ob = o_pool.tile([128, 512], f32, tag="o")
                        nc.vector.tensor_copy(ob[:], ps[:])
                        nc.sync.dma_start(
                            outT[m * 128:(m + 1) * 128,
                                 tch * 512:(tch + 1) * 512], ob[:])
    nc.finalize()
    return nc


def _prep_inputs(x, w_in, w_out):
    """Build per-core input maps (host-side sharding)."""
    x = np.ascontiguousarray(x, dtype=np.float32)
    w_in = np.ascontiguousarray(w_in, dtype=np.float32)
    w_out = np.ascontiguousarray(w_out, dtype=np.float32)

    tri = np.triu(np.ones((128, 128), dtype=np.float32))  # 1 where k <= q
    in_maps = []
    for c in range(N_CORES):
        b, g = c // 2, c % 2
        heads = [8 * g + h for h in range(HPC)]
        xTb = np.ascontiguousarray(x[b].T)                       # [D, S]
        # wqkT: cols i*128 -> q rows of heads (8g+2i, 8g+2i+1); then k pairs
        qcols, kcols = [], []
        for i in range(NPAIRS):
            hA, hB = heads[2 * i], heads[2 * i + 1]
            qcols.append(w_in[hA * HD:(hA + 1) * HD, :])
            qcols.append(w_in[hB * HD:(hB + 1) * HD, :])
            kcols.append(w_in[D + hA * HD:D + (hA + 1) * HD, :])
            kcols.append(w_in[D + hB * HD:D + (hB + 1) * HD, :])
        wqkT = np.ascontiguousarray(
            np.concatenate(qcols + kcols, axis=0).T)             # [D, 1024]
        wvT = np.ascontiguousarray(np.concatenate(
            [w_in[2 * D + h * HD:2 * D + (h + 1) * HD, :] for h in heads],
            axis=0).T)                                           # [D, 512]
        woT = np.ascontiguousarray(
            w_out[g * EHALF:(g + 1) * EHALF, :].T)               # [D, 512]
        in_maps.append({
            "xT": xTb, "wqkT": wqkT, "wvT": wvT, "woT": woT, "tri": tri,
        })
    return in_maps


def kernel(x, w_in, w_out):
    global _PROG
    from concourse.bass_utils import run_bass_kernel_spmd

    if _PROG is None:
        _PROG = _build_program()
    in_maps = _prep_inputs(x, w_in, w_out)
    res = run_bass_kernel_spmd(_PROG, in_maps, list(range(N_CORES)))

    out = np.empty((B, S, D), dtype=np.float32)
    for c in range(N_CORES):
        b, g = c // 2, c % 2
        out[b, :, g * EHALF:(g + 1) * EHALF] = res.results[c]["outT"].T
    return out



# revision 4
# speedup vs baseline: 1.3943x; 1.3943x over previous
"""Causal self-attention on 8 Trainium2 NeuronCores (Bass/Tile), v2.

Problem: x[4, 2048, 1024], w_in[3072, 1024], w_out[1024, 1024], 16 heads.
    qkv = x @ w_in.T ; per-(b,h) causal softmax attention ; out = y @ w_out.T

Sharding (SPMD — one program, per-core input data):
    core c  ->  batch b = c // 2, head-group g = c % 2 (heads 8g .. 8g+7).
    Each core projects q/k/v for its 8 heads of its batch and runs causal
    attention for them.  The pair (2b, 2b+1) AllGathers normalized yT in
    [128, 512] bf16 chunks (per head-pair per q-chunk) so the exchange
    pipelines with attention; each core then computes the output projection
    for half the output features (host-selected via woT) over all 2048
    tokens of its batch.  The host concatenates.

v2 changes vs the 492 us baseline:
  * all matmul operands bf16 (numpy error sim: 4.2e-3 rel err vs the 2e-2
    gate); halves DMA bytes and removes the fp32r ap<256 penalty.
  * input DMA quarter-chunked and ordered (wv, then xT quarters) so the
    first v-proj matmul starts ~2 us in (was a 37 us startup stall).
  * attention inner loop software-pipelined: scores for tile kt+1 are
    issued before AV of tile kt, so the in-order PE does not sit behind
    the ACT engine's exp.
  * the next pair's q/k projection matmuls (and, for the last pair,
    pass A of the output projection) are interleaved into the current
    pair's ACT-bound attention window to fill PE slack.
  * AllGather per (pair, q-chunk) in bf16 — 16 x 128 KiB chunks that
    pipeline with attention instead of 4 x 1 MiB fp32 lumps; gathered
    chunks are DMA'd back to SBUF as they land.
  * output projection split: pass A contracts over head-pairs 0,1 early;
    pass B (pairs 2,3) runs at the tail gated only on the late gathers,
    then a DVE add joins the halves.
  * softmax normalize broadcast moved from a PE matmul to GpSimd
    partition_broadcast (saves ~16k PE rows and a PSUM pool).
"""

import sys

for _p in ("/opt/trn_rl_repo",):
    if _p not in sys.path:
        sys.path.insert(0, _p)

import numpy as np

B, S, D = 4, 2048, 1024
H, HD = 16, 64
N_CORES = 8
HPC = 8            # heads per core
NPAIRS = HPC // 2  # head pairs per core
QC = S // 512      # q-chunks per head
TT = S // 128      # token tiles
DT = D // 128      # feature (d) tiles
EHALF = D // 2     # output features per core

_PROG = None       # cached compiled program


def _build_program():
    from concourse import bacc
    import concourse.tile as tile
    import concourse.mybir as mybir
    from contextlib import ExitStack

    f32 = mybir.dt.float32
    bf16 = mybir.dt.bfloat16
    AF = mybir.ActivationFunctionType
    OP = mybir.AluOpType

    nc = bacc.Bacc("TRN2", target_bir_lowering=False, debug=False,
                   num_devices=N_CORES)

    xT = nc.dram_tensor("xT", [D, S], bf16, kind="ExternalInput").ap()
    wqkT = nc.dram_tensor("wqkT", [D, NPAIRS * 256], bf16,
                          kind="ExternalInput").ap()
    wvT = nc.dram_tensor("wvT", [D, HPC * HD], bf16, kind="ExternalInput").ap()
    woT = nc.dram_tensor("woT", [D, EHALF], bf16, kind="ExternalInput").ap()
    tri = nc.dram_tensor("tri", [128, 128], bf16, kind="ExternalInput").ap()
    outT = nc.dram_tensor("outT", [EHALF, S], f32, kind="ExternalOutput").ap()

    # per-(pair, qc) gather chunks
    y_pq = [[nc.dram_tensor(f"y_pq{i}_{qc}", [128, 512], bf16)
             for qc in range(QC)] for i in range(NPAIRS)]
    y_gat = [[nc.dram_tensor(f"y_gat{i}_{qc}", [2, 128, 512], bf16)
              for qc in range(QC)] for i in range(NPAIRS)]
    RG = [[0, 1], [2, 3], [4, 5], [6, 7]]

    with tile.TileContext(nc) as tc:
        def mm(out, lhsT, rhs, start, stop):
            nc.tensor.matmul(out, lhsT, rhs, start=start, stop=stop)

        with ExitStack() as perm:
            const_pool = perm.enter_context(tc.tile_pool(name="const", bufs=1))
            v_pool = perm.enter_context(tc.tile_pool(name="vsb", bufs=TT))
            xt_pool = perm.enter_context(tc.tile_pool(name="xtsb", bufs=DT))
            ys_pool = perm.enter_context(
                tc.tile_pool(name="yssb", bufs=2 * NPAIRS * QC))
            wo_pool = perm.enter_context(tc.tile_pool(name="wosb", bufs=DT))
            oa_pool = perm.enter_context(
                tc.tile_pool(name="oasb", bufs=4 * QC))
            mm_ps = perm.enter_context(
                tc.tile_pool(name="mmps", bufs=2, space="PSUM"))

            tri_sb = const_pool.tile([128, 128], bf16, tag="tri")
            ones_sb = const_pool.tile([128, HPC], bf16, tag="ones")

            nc.sync.dma_start(tri_sb[:], tri[:])
            nc.gpsimd.memset(ones_sb[:], 1.0)

            # ---- input DMA, ordered for earliest first matmul ----
            with ExitStack() as big:
                wv_pool = big.enter_context(tc.tile_pool(name="wvsb",
                                                         bufs=DT))
                qk_pool = big.enter_context(tc.tile_pool(name="qksb", bufs=4))
                wqk_pool = big.enter_context(
                    tc.tile_pool(name="wqksb", bufs=2 * DT))
                p_pool = big.enter_context(tc.tile_pool(name="psb", bufs=3))
                n_pool = big.enter_context(tc.tile_pool(name="nsb", bufs=4))
                o_pool = big.enter_context(tc.tile_pool(name="osb", bufs=2))
                sc_ps = big.enter_context(
                    tc.tile_pool(name="scps", bufs=2, space="PSUM"))
                y_ps = big.enter_context(
                    tc.tile_pool(name="yps", bufs=2, space="PSUM"))

                wv_sb = [wv_pool.tile([128, HPC * HD], bf16, tag="wv",
                                      name=f"wv{d}") for d in range(DT)]
                for d in range(DT):
                    nc.sync.dma_start(wv_sb[d][:],
                                      wvT[d * 128:(d + 1) * 128, :])

                xt_sb = [xt_pool.tile([128, S], bf16, tag="xt", name=f"xt{d}")
                         for d in range(DT)]
                for quarter in range(4):
                    sl = slice(quarter * 512, (quarter + 1) * 512)
                    for d in range(DT):
                        nc.sync.dma_start(xt_sb[d][:, sl],
                                          xT[d * 128:(d + 1) * 128, sl])

                wo_sb = [wo_pool.tile([128, EHALF], bf16, tag="wo",
                                      name=f"wo{d}") for d in range(DT)]
                for d in range(DT):
                    nc.sync.dma_start(wo_sb[d][:],
                                      woT[d * 128:(d + 1) * 128, :])

                # v_sb[t]: [128, 8*65] — per head 64 v-cols + a ones column
                v_sb = [v_pool.tile([128, HPC * (HD + 1)], bf16, tag="v",
                                    name=f"v{t}") for t in range(TT)]
                # gathered yT chunks: global feature-tile f = slot*4 + pair
                ys_sb = [[ys_pool.tile([128, 512], bf16, tag="ys",
                                       name=f"ys{f}_{qc}")
                          for qc in range(QC)] for f in range(2 * NPAIRS)]
                # pass-A partial out tiles
                oa_sb = [[oa_pool.tile([128, 512], f32, tag="oa",
                                       name=f"oa{m}_{t}")
                          for t in range(QC)] for m in range(EHALF // 128)]

                # ---- v projection: v[t, e] accumulated over d ----
                for t in range(TT):
                    ps = mm_ps.tile([128, 512], f32, tag="mm")
                    for d in range(DT):
                        mm(ps[:], xt_sb[d][:, t * 128:(t + 1) * 128],
                           wv_sb[d][:], start=(d == 0), stop=(d == DT - 1))
                    vdst = v_sb[t][:].rearrange(
                        "p (h e) -> p h e", h=HPC)[:, :, 0:HD]
                    vsrc = ps[:].rearrange("p (h e) -> p h e", h=HPC)
                    nc.vector.tensor_copy(vdst, vsrc)
                    nc.vector.tensor_copy(
                        v_sb[t][:].rearrange(
                            "p (h e) -> p h e", h=HPC)[:, :, HD:HD + 1],
                        ones_sb[:].unsqueeze(-1))

                # ---- per-pair state + helpers ----
                wqk_sb = [None] * NPAIRS
                qk_sb = [None] * NPAIRS

                def fetch_wqk(i):
                    wqk_sb[i] = [wqk_pool.tile([128, 256], bf16, tag="wqk",
                                               name=f"wqk{i}_{d}")
                                 for d in range(DT)]
                    for d in range(DT):
                        nc.sync.dma_start(
                            wqk_sb[i][d][:],
                            wqkT[d * 128:(d + 1) * 128,
                                 i * 256:(i + 1) * 256])

                def proj_qk_chunk(i, which, qc):
                    """One q (which=0) or k (which=1) projection chunk:
                    8 matmuls + 1 PSUM->SBUF bf16 copy."""
                    if qk_sb[i] is None:
                        qk_sb[i] = (
                            qk_pool.tile([128, S], bf16, tag="qk",
                                         name=f"q{i}"),
                            qk_pool.tile([128, S], bf16, tag="qk",
                                         name=f"k{i}"))
                    dest = qk_sb[i][which]
                    ps = mm_ps.tile([128, 512], f32, tag="mm")
                    for d in range(DT):
                        mm(ps[:],
                           wqk_sb[i][d][:, which * 128:(which + 1) * 128],
                           xt_sb[d][:, qc * 512:(qc + 1) * 512],
                           start=(d == 0), stop=(d == DT - 1))
                    nc.vector.tensor_copy(dest[:, qc * 512:(qc + 1) * 512],
                                          ps[:])

                def passA_tile(m, t):
                    ps = mm_ps.tile([128, 512], f32, tag="mm")
                    for n, f in enumerate((0, 1, 4, 5)):
                        mm(ps[:], wo_sb[f][:, m * 128:(m + 1) * 128],
                           ys_sb[f][t][:], start=(n == 0), stop=(n == 3))
                    nc.vector.tensor_copy(oa_sb[m][t][:], ps[:])

                def passB_tile(m, t):
                    ps = mm_ps.tile([128, 512], f32, tag="mm")
                    for n, f in enumerate((2, 3, 6, 7)):
                        mm(ps[:], wo_sb[f][:, m * 128:(m + 1) * 128],
                           ys_sb[f][t][:], start=(n == 0), stop=(n == 3))
                    ob = o_pool.tile([128, 512], f32, tag="o")
                    nc.vector.tensor_add(ob[:], ps[:], oa_sb[m][t][:])
                    nc.sync.dma_start(
                        outT[m * 128:(m + 1) * 128,
                             t * 512:(t + 1) * 512], ob[:])

                def emit_scores(i, qc, kt):
                    """Scores + exp + mask for one (kt, qc) tile; returns
                    (prob tile, lo)."""
                    q_sb, k_sb = qk_sb[i]
                    j = kt - 4 * qc
                    lo = max(0, j) * 128
                    sc = sc_ps.tile([128, 1024], f32, tag="sc")
                    pt = p_pool.tile([128, 1024], bf16, tag="p")
                    for h in range(2):
                        mm(sc[:, h * 512 + lo:(h + 1) * 512],
                           k_sb[h * 64:(h + 1) * 64,
                                kt * 128:(kt + 1) * 128],
                           q_sb[h * 64:(h + 1) * 64,
                                qc * 512 + lo:(qc + 1) * 512],
                           start=True, stop=True)
                    src = sc[:].rearrange("p (s c) -> p s c", s=2)[
                        :, :, lo:512]
                    dst = pt[:].rearrange("p (s c) -> p s c", s=2)[
                        :, :, lo:512]
                    nc.scalar.activation(dst, src, AF.Exp, scale=0.125)
                    if j >= 0:   # mask the diagonal band
                        for h in range(2):
                            band = pt[:, h * 512 + lo:h * 512 + lo + 128]
                            nc.vector.tensor_mul(band, band, tri_sb[:])
                    return pt, lo

                def emit_av(i, kt, nkt, pt, lo, yps):
                    for h in range(2):
                        hl = 2 * i + h
                        mm(yps[h][:, lo:512],
                           v_sb[kt][:, hl * 65:hl * 65 + 65],
                           pt[:, h * 512 + lo:(h + 1) * 512],
                           start=(kt == 0), stop=(kt == nkt - 1))

                def normalize_and_ship(i, qc, yps):
                    """Normalize both heads of (pair i, chunk qc), ship the
                    [128, 512] bf16 chunk and kick its AllGather; fetch the
                    gathered slots back into SBUF."""
                    ych = n_pool.tile([128, 512], bf16, tag="ych")
                    for h in range(2):
                        ysc = n_pool.tile([65, 512], f32, tag="ysc")
                        nc.vector.tensor_copy(ysc[:], yps[h][:])
                        srow = n_pool.tile([1, 512], f32, tag="srow")
                        nc.sync.dma_start(srow[:], ysc[64:65, :])
                        rcp = n_pool.tile([1, 512], f32, tag="rcp")
                        nc.vector.reciprocal_approx_fast(
                            out=rcp[:], in_=srow[:])
                        rbb = n_pool.tile([64, 512], f32, tag="rbb")
                        nc.gpsimd.partition_broadcast(rbb[:], rcp[:])
                        nc.vector.tensor_mul(ych[h * 64:(h + 1) * 64, :],
                                             ysc[0:64, :], rbb[:])
                    nc.sync.dma_start(y_pq[i][qc][:], ych[:])
                    nc.gpsimd.collective_compute(
                        "AllGather", OP.bypass,
                        replica_groups=RG,
                        ins=[y_pq[i][qc][:]],
                        outs=[y_gat[i][qc][:]])
                    for slot in range(2):
                        nc.sync.dma_start(ys_sb[slot * NPAIRS + i][qc][:],
                                          y_gat[i][qc][slot])

                # ---------- pair pipeline ----------
                fetch_wqk(0)
                for which in range(2):
                    for qc in range(QC):
                        proj_qk_chunk(0, which, qc)
                fetch_wqk(1)

                for i in range(NPAIRS):
                    if i + 1 < NPAIRS:
                        filler = [("proj", i + 1, which, qc)
                                  for which in range(2) for qc in range(QC)]
                    else:
                        filler = [("passA", m, t)
                                  for m in range(4) for t in range(QC)]
                    fidx = 0

                    def run_filler():
                        nonlocal fidx
                        if fidx < len(filler):
                            it = filler[fidx]
                            fidx += 1
                            if it[0] == "proj":
                                proj_qk_chunk(it[1], it[2], it[3])
                            else:
                                passA_tile(it[1], it[2])

                    for qc in range(QC):
                        nkt = 4 * qc + 4
                        yps = [y_ps.tile([65, 512], f32, tag="yt",
                                         name=f"yps{i}_{qc}_{h}")
                               for h in range(2)]
                        prev = None
                        for kt in range(nkt):
                            cur = emit_scores(i, qc, kt)
                            if prev is not None:
                                emit_av(i, kt - 1, nkt, prev[0], prev[1],
                                        yps)
                            prev = cur
                            if kt % 3 == 2:
                                run_filler()
                        emit_av(i, nkt - 1, nkt, prev[0], prev[1], yps)
                        normalize_and_ship(i, qc, yps)
                    while fidx < len(filler):
                        run_filler()
                    if i + 2 < NPAIRS:
                        fetch_wqk(i + 2)

                # ---------- output projection pass B + join ----------
                for t in range(QC):
                    for m in range(4):
                        passB_tile(m, t)

    nc.finalize()
    return nc


def _prep_inputs(x, w_in, w_out):
    """Build per-core input maps (host-side sharding), bf16."""
    import ml_dtypes
    bf = ml_dtypes.bfloat16

    x = np.asarray(x, dtype=np.float32)
    w_in = np.asarray(w_in, dtype=np.float32)
    w_out = np.asarray(w_out, dtype=np.float32)

    tri = np.triu(np.ones((128, 128), dtype=np.float32)).astype(bf)
    in_maps = []
    for c in range(N_CORES):
        b, g = c // 2, c % 2
        heads = [8 * g + h for h in range(HPC)]
        xTb = np.ascontiguousarray(x[b].T.astype(bf))            # [D, S]
        # wqkT: per pair i cols [256i:256i+128] = q rows of heads
        # (8g+2i, 8g+2i+1); cols [256i+128:256i+256] = k rows.
        pcols = []
        for i in range(NPAIRS):
            hA, hB = heads[2 * i], heads[2 * i + 1]
            pcols += [w_in[hA * HD:(hA + 1) * HD, :],
                      w_in[hB * HD:(hB + 1) * HD, :],
                      w_in[D + hA * HD:D + (hA + 1) * HD, :],
                      w_in[D + hB * HD:D + (hB + 1) * HD, :]]
        wqkT = np.ascontiguousarray(
            np.concatenate(pcols, axis=0).T.astype(bf))          # [D, 1024]
        wvT = np.ascontiguousarray(np.concatenate(
            [w_in[2 * D + h * HD:2 * D + (h + 1) * HD, :] for h in heads],
            axis=0).T.astype(bf))                                # [D, 512]
        woT = np.ascontiguousarray(
            w_out[g * EHALF:(g + 1) * EHALF, :].T.astype(bf))    # [D, 512]
        in_maps.append({
            "xT": xTb, "wqkT": wqkT, "wvT": wvT, "woT": woT, "tri": tri,
        })
    return in_maps


def kernel(x, w_in, w_out):
    global _PROG
    from concourse.bass_utils import run_bass_kernel_spmd

    if _PROG is None:
        _PROG = _build_program()
    in_maps = _prep_inputs(x, w_in, w_out)
    res = run_bass_kernel_spmd(_PROG, in_maps, list(range(N_CORES)))

    out = np.empty((B, S, D), dtype=np.float32)
    for c in range(N_CORES):
        b, g = c // 2, c % 2
        out[b, :, g * EHALF:(g + 1) * EHALF] = res.results[c]["outT"].T
    return out
